# revision 6
# baseline (speedup 1.0000x reference)
"""Multi-head attention (B=2, S=2048, D=1024, H=16, causal) on 8 TRN2 NeuronCores.

Sharding: 8 cores = 2 batches x 4 head-groups (4 heads each).  Each core
computes the QKV projections for its head slice, causal attention for its 4
heads, and the partial output projection (input-dim slice of Wo).  The
all-reduce over head-groups happens at gather time on the host (sum of 4
partials per batch), which is the "all-reduce after the output projection"
of a tensor-parallel split.

Everything on device works in token-transposed layout ([feature, token]) so
no on-device transposes are needed:
  scores^T[kv, q] = K_projT_tile^T @ Q_projT   (K = dh = 64)
  P = exp(scores^T)  (no max subtraction needed: scores ~ N(0,1), |s| < ~7)
  out^T[dh(+1), q] = [V | ones]^T @ P          (ones column -> softmax denom)
  partial^T[dmodel, tok] = WoT_slice^T @ attn_out^T
"""

import math
import os

import numpy as np
import ml_dtypes

_BF16 = ml_dtypes.bfloat16

B, S, D = 2, 2048, 1024
H, DH = 16, 64
NCORES = 8
GRP = 4  # heads per core
KT = D // 128  # 8 k-tiles over d_model
NQ = 512  # q tile width (free dim of score tiles)
NKV = 128  # kv tile width (partition dim of score tiles)
QTILES = S // NQ  # 4
KVTILES = S // NKV  # 16

# module-level stash so test.py can read profiling info
last_results = None

_programs = {}


def _build_program(causal: bool):
    import concourse.bass as bass
    import concourse.mybir as mybir
    import concourse.tile as tile
    from concourse import bacc

    f32 = mybir.dt.float32
    bf16 = mybir.dt.bfloat16
    Exp = mybir.ActivationFunctionType.Exp

    nc = bacc.Bacc(
        "TRN2",
        target_bir_lowering=False,
        debug=False,
        enable_asserts=False,
        num_devices=NCORES,
    )

    qT = nc.dram_tensor("qT", [D, S], bf16, kind="ExternalInput").ap()
    kT = nc.dram_tensor("kT", [D, S], bf16, kind="ExternalInput").ap()
    vT = nc.dram_tensor("vT", [D, S], bf16, kind="ExternalInput").ap()
    wqT = nc.dram_tensor("wqT", [D, 256], bf16, kind="ExternalInput").ap()
    wkT = nc.dram_tensor("wkT", [D, 256], bf16, kind="ExternalInput").ap()
    wvT = nc.dram_tensor("wvT", [D, 256], bf16, kind="ExternalInput").ap()
    woT = nc.dram_tensor("woT", [256, D], bf16, kind="ExternalInput").ap()
    if not causal:
        maskT = nc.dram_tensor("maskT", [S, S], bf16, kind="ExternalInput").ap()
    out = nc.dram_tensor("out", [D, S], f32, kind="ExternalOutput").ap()

    with tile.TileContext(nc) as tc:
        with (
            tc.tile_pool(name="persist", bufs=1) as sb,
            tc.tile_pool(name="stream", bufs=12) as stream,
            tc.tile_pool(name="proj_ps", bufs=2, space="PSUM") as ppool,
            tc.tile_pool(name="score_ps", bufs=3, space="PSUM") as spool,
            tc.tile_pool(name="av_ps", bufs=2, space="PSUM") as avpool,
            tc.tile_pool(name="p_sb", bufs=6) as pbuf,
            tc.tile_pool(name="r_sb", bufs=3) as rpool,
            tc.tile_pool(name="m_sb", bufs=4) as mpool,
            tc.tile_pool(name="o_sb", bufs=4) as opool,
        ):
            # ---- persistent SBUF tensors ----
            wq_sb = sb.tile([128, KT, 256], bf16)
            wk_sb = sb.tile([128, KT, 256], bf16)
            wv_sb = sb.tile([128, KT, 256], bf16)
            wo_sb = sb.tile([64, GRP, D], bf16)
            qproj = sb.tile([128, 2, S], bf16)
            kproj = sb.tile([128, 2, S], bf16)
            vproj = sb.tile([128, KVTILES, GRP, 66], bf16)
            attn = sb.tile([64, GRP, S], bf16)

            for kt in range(KT):
                nc.sync.dma_start(wq_sb[:, kt, :], wqT[128 * kt : 128 * kt + 128, :])
                nc.sync.dma_start(wk_sb[:, kt, :], wkT[128 * kt : 128 * kt + 128, :])
                nc.sync.dma_start(wv_sb[:, kt, :], wvT[128 * kt : 128 * kt + 128, :])
            for h in range(GRP):
                nc.sync.dma_start(wo_sb[:, h, :], woT[64 * h : 64 * h + 64, :])

            # ones columns at index 0 and 65 of vproj (V lands in cols 1..64)
            nc.vector.memset(vproj[:], 1.0)

            if causal:
                # mask_sb[:, d, :]:  keep (1.0) where q_local - kv_local >= 128*d
                mask_sb = sb.tile([128, 4, NQ], bf16)
                nc.vector.memset(mask_sb[:], 1.0)
                for d in range(4):
                    nc.gpsimd.affine_select(
                        out=mask_sb[:, d, :],
                        in_=mask_sb[:, d, :],
                        compare_op=mybir.AluOpType.is_ge,
                        fill=0.0,
                        base=-128 * d,
                        pattern=[[1, NQ]],
                        channel_multiplier=-1,
                    )

            # ---- Q / K projections (transposed): proj^T[256, S] ----
            for w_sb, x_dram, proj in ((wq_sb, qT, qproj), (wk_sb, kT, kproj)):
                xt = []
                for kt in range(KT):
                    t = stream.tile([128, S], bf16, tag="xT")
                    nc.sync.dma_start(t[:], x_dram[128 * kt : 128 * kt + 128, :])
                    xt.append(t)
                for m2 in range(2):
                    for n in range(QTILES):
                        ps = ppool.tile([128, NQ], f32, tag="proj")
                        for kt in range(KT):
                            nc.tensor.matmul(
                                ps[:],
                                w_sb[:, kt, 128 * m2 : 128 * m2 + 128],
                                xt[kt][:, NQ * n : NQ * n + NQ],
                                start=(kt == 0),
                                stop=(kt == KT - 1),
                            )
                        nc.vector.tensor_copy(proj[:, m2, NQ * n : NQ * n + NQ], ps[:])

            # ---- V projection (normal layout): v_proj[tok, 256] ----
            vt = []
            for kt in range(KT):
                t = stream.tile([128, S], bf16, tag="xT")
                nc.sync.dma_start(t[:], vT[128 * kt : 128 * kt + 128, :])
                vt.append(t)
            for mt in range(KVTILES):
                ps = ppool.tile([128, 256], f32, tag="proj")
                for kt in range(KT):
                    nc.tensor.matmul(
                        ps[:],
                        vt[kt][:, 128 * mt : 128 * mt + 128],
                        wv_sb[:, kt, :],
                        start=(kt == 0),
                        stop=(kt == KT - 1),
                    )
                nc.vector.tensor_copy(
                    vproj[:, mt, :, 1:65],
                    ps[:].rearrange("p (h d) -> p h d", h=GRP),
                )

            # ---- attention per head ----
            for h in range(GRP):
                h2, hp = h // 2, 64 * (h % 2)
                for j in range(QTILES):
                    av = avpool.tile([65, NQ], f32, tag="av")
                    ktiles = 4 * j + 4 if causal else KVTILES
                    for t in range(ktiles):
                        sp = spool.tile([128, NQ], f32, tag="sc")
                        nc.tensor.matmul(
                            sp[:],
                            kproj[hp : hp + 64, h2, 128 * t : 128 * t + 128],
                            qproj[hp : hp + 64, h2, NQ * j : NQ * j + NQ],
                            start=True,
                            stop=True,
                        )
                        p = pbuf.tile([128, NQ], bf16, tag="p")
                        nc.scalar.activation(p[:], sp[:], Exp)
                        if causal:
                            if t >= 4 * j:
                                nc.vector.tensor_mul(p[:], p[:], mask_sb[:, t - 4 * j, :])
                        else:
                            mt_t = mpool.tile([128, NQ], bf16, tag="mt")
                            nc.sync.dma_start(
                                mt_t[:],
                                maskT[128 * t : 128 * t + 128, NQ * j : NQ * j + NQ],
                            )
                            nc.vector.tensor_mul(p[:], p[:], mt_t[:])
                        nc.tensor.matmul(
                            av[:],
                            vproj[:, t, h, 1:66],
                            p[:],
                            start=(t == 0),
                            stop=(t == ktiles - 1),
                        )
                    # normalize: attn[:, h, q] = av[0:64, q] / av[64, q]
                    # (reciprocal of a single row is slow on DVE -> DMA-reshape
                    #  the 512 sums to [128, 4], recip there, reshape back to
                    #  partition 0, then gpsimd-broadcast to 64 partitions)
                    rs = rpool.tile([65, NQ], f32, tag="rs")
                    nc.vector.tensor_copy(rs[64:65, :], av[64:65, :])
                    rq = rpool.tile([128, 4], f32, tag="rq")
                    nc.sync.dma_start(rq[:], rs[64:65, :])
                    rqr = rpool.tile([128, 4], f32, tag="rqr")
                    nc.vector.reciprocal(rqr[:], rq[:])
                    rr = rpool.tile([1, NQ], f32, tag="rr")
                    nc.sync.dma_start(rr[:], rqr[:])
                    rb = rpool.tile([64, NQ], f32, tag="rb")
                    nc.gpsimd.partition_broadcast(rb[:], rr[0:1, :], channels=64)
                    nc.vector.tensor_mul(
                        attn[:, h, NQ * j : NQ * j + NQ], av[0:64, :], rb[:]
                    )

            # ---- output projection partial^T[1024, S] ----
            for m in range(D // 128):
                for n in range(QTILES):
                    ps = ppool.tile([128, NQ], f32, tag="proj")
                    for h in range(GRP):
                        nc.tensor.matmul(
                            ps[:],
                            wo_sb[:, h, 128 * m : 128 * m + 128],
                            attn[:, h, NQ * n : NQ * n + NQ],
                            start=(h == 0),
                            stop=(h == GRP - 1),
                        )
                    ot = opool.tile([128, NQ], f32, tag="ot")
                    nc.vector.tensor_copy(ot[:], ps[:])
                    nc.sync.dma_start(
                        out[128 * m : 128 * m + 128, NQ * n : NQ * n + NQ], ot[:]
                    )

    nc.compile()
    return nc


def _get_program(causal: bool):
    if causal not in _programs:
        _programs[causal] = _build_program(causal)
    return _programs[causal]


def kernel(query, key, value, mask, Wq, Wk, Wv, Wo):
    global last_results
    from concourse.bass_utils import run_bass_kernel_spmd

    query = np.asarray(query, dtype=np.float32)
    key = np.asarray(key, dtype=np.float32)
    value = np.asarray(value, dtype=np.float32)
    Wq = np.asarray(Wq, dtype=np.float32)
    Wk = np.asarray(Wk, dtype=np.float32)
    Wv = np.asarray(Wv, dtype=np.float32)
    Wo = np.asarray(Wo, dtype=np.float32)
    m2d = np.asarray(mask).reshape(S, S).astype(bool)

    causal = bool(np.array_equal(m2d, np.tril(np.ones((S, S), dtype=bool))))
    nc = _get_program(causal)

    scale = 1.0 / math.sqrt(DH)
    # transposed inputs / weights, bf16
    WqT = np.ascontiguousarray((Wq * scale).T).astype(_BF16)  # [D, D] cols=out dims
    WkT = np.ascontiguousarray(Wk.T).astype(_BF16)
    WvT = np.ascontiguousarray(Wv.T).astype(_BF16)
    WoT = np.ascontiguousarray(Wo.T).astype(_BF16)  # rows = head-concat dims
    xT = {
        "qT": [np.ascontiguousarray(query[b].T).astype(_BF16) for b in range(B)],
        "kT": [np.ascontiguousarray(key[b].T).astype(_BF16) for b in range(B)],
        "vT": [np.ascontiguousarray(value[b].T).astype(_BF16) for b in range(B)],
    }
    if not causal:
        maskT = np.ascontiguousarray(m2d.T).astype(_BF16)

    in_maps = []
    for c in range(NCORES):
        b, g = c // 4, c % 4
        sl = slice(256 * g, 256 * g + 256)
        im = {
            "qT": xT["qT"][b],
            "kT": xT["kT"][b],
            "vT": xT["vT"][b],
            "wqT": np.ascontiguousarray(WqT[:, sl]),
            "wkT": np.ascontiguousarray(WkT[:, sl]),
            "wvT": np.ascontiguousarray(WvT[:, sl]),
            "woT": np.ascontiguousarray(WoT[sl, :]),
        }
        if not causal:
            im["maskT"] = maskT
        in_maps.append(im)

    trace = os.environ.get("KERNEL_PROFILE", "") == "1"
    res = run_bass_kernel_spmd(nc, in_maps, list(range(NCORES)), trace=trace)
    last_results = res

    outp = np.empty((B, S, D), dtype=np.float32)
    for b in range(B):
        acc = res.results[4 * b]["out"].astype(np.float32)
        for g in range(1, 4):
            acc = acc + res.results[4 * b + g]["out"]
        outp[b] = acc.T
    return outp
